# revision 27
# baseline (speedup 1.0000x reference)
"""Batch-parallel attention kernel for 8 TRN2 NeuronCores.

Problem: B=16, S=2048, D=128 full (non-causal) attention, fp32 I/O.
Sharding: batch dim across 8 cores (2 batches/core), no collectives.

Per-core layout trick: everything is computed in "transposed score" space
S^T[k, q] so that no on-device transposes are needed:
  - matmul1: S^T[k,q] = (K^T)[d,k]^T-stationary @ (Q^T)[d,q]-moving,
    contraction over d=128 partitions. Q^T/K^T are prepared on host.
  - ScalarE: expS^T = exp(scale * S^T) PSUM->SBUF (bf16), no max
    subtraction (scores are ~N(0,1); max over dataset ~7.5 -> exp fine).
    ScalarE is the bottleneck engine (1 elem/cycle/lane + ~300ns fixed
    cost per instruction), so scores are grouped 4+2 k-tiles deep in
    PSUM and exp'd in the largest calls the 8 PSUM banks allow.
  - matmul2: out[q, 0:129] = sum_k expS^T[k,q]^T-stationary @ V_aug[k,:]
    where V_aug = [V | ones]; column 128 accumulates the softmax
    denominator exactly in fp32 PSUM. Accumulators are packed two per
    PSUM bank ([128, 2, 129] tiles) to free banks for the score groups.
  - VectorE: reciprocal of the denominator column + per-partition
    tensor_scalar multiply -> normalized out tile, DMA'd out natively.

PSUM budget: 4 (score group A) + 2 (score group B) + 2 (accumulators).
"""

import os

import ml_dtypes
import numpy as np

import concourse.bass as bass
import concourse.mybir as mybir
import concourse.tile as tile
from concourse import bacc
from concourse.bass_utils import run_bass_kernel_spmd

B, S, D = 16, 2048, 128
N_CORES = 8
BPC = B // N_CORES          # batches per core
DA = D + 1                  # V augmented with ones column
QCHUNK = 512                # q processed per inner pipeline chunk
N_QC = S // QCHUNK          # 4
N_KT = S // 128             # 16 k-tiles
KT_GROUPS = [(k, 2) for k in range(0, 16, 2)]
SCALE = 1.0 / float(np.sqrt(D))

BF16 = mybir.dt.bfloat16
F32 = mybir.dt.float32

TRACE = bool(os.environ.get("BASS_KERNEL_TRACE"))
LAST_RESULTS = None

_CACHE = {}


def _build():
    nc = bacc.Bacc("TRN2", target_bir_lowering=False, debug=False)

    qT = nc.dram_tensor("qT", [BPC, D, S], BF16, kind="ExternalInput").ap()
    kT = nc.dram_tensor("kT", [BPC, D, S], BF16, kind="ExternalInput").ap()
    vA = nc.dram_tensor("vA", [BPC, S, DA], BF16, kind="ExternalInput").ap()
    out = nc.dram_tensor("out", [BPC, S, D], F32, kind="ExternalOutput").ap()

    with tile.TileContext(nc) as tc:
        with (
            tc.tile_pool(name="qk", bufs=2) as qk_pool,
            tc.tile_pool(name="vp", bufs=2) as v_pool,
            tc.tile_pool(name="warm", bufs=1) as warm_pool,
            tc.tile_pool(name="pexp", bufs=6) as p_pool,
            tc.tile_pool(name="outs", bufs=12) as o_pool,
            tc.tile_pool(name="psum_s", bufs=1, space="PSUM") as psum_s,
            tc.tile_pool(name="psum_acc", bufs=1, space="PSUM") as psum_acc,
        ):
            # Pull the ~2.7us exp table load to t=0 so it overlaps the input
            # DMAs instead of stalling the first real exp.
            wtile = warm_pool.tile([128, 1], F32)
            nc.vector.memset(wtile, 0.0)
            nc.scalar.activation(
                wtile, wtile, mybir.ActivationFunctionType.Exp
            )

            QS = S // 4
            batch_tiles = {}

            def load_batch(b):
                # Quarter-tiles with independent DMAs so the first matmul
                # only waits on one 128KB transfer; loads spread over three
                # engines' DMA queues, first-needed quarters first.
                kT_sb = [qk_pool.tile([128, QS], BF16, tag=f"kT{h}",
                                      name=f"kT{h}") for h in range(4)]
                qT_sb = [qk_pool.tile([128, QS], BF16, tag=f"qT{h}",
                                      name=f"qT{h}") for h in range(4)]
                v_sb = [v_pool.tile([128, N_KT // 2, DA], BF16, tag=f"v{h}",
                                    name=f"v{h}") for h in range(2)]
                nc.sync.dma_start(out=kT_sb[0], in_=kT[b][:, 0:QS])
                nc.sync.dma_start(out=qT_sb[0], in_=qT[b][:, 0:QS])
                nc.gpsimd.dma_start(out=qT_sb[1], in_=qT[b][:, QS : 2 * QS])
                nc.sync.dma_start(out=kT_sb[1], in_=kT[b][:, QS : 2 * QS])
                nc.gpsimd.dma_start(
                    out=v_sb[0],
                    in_=vA[b][0 : S // 2].rearrange("(t p) d -> p t d", p=128),
                )
                nc.sync.dma_start(out=kT_sb[2], in_=kT[b][:, 2 * QS : 3 * QS])
                nc.gpsimd.dma_start(out=qT_sb[2], in_=qT[b][:, 2 * QS : 3 * QS])
                nc.sync.dma_start(out=kT_sb[3], in_=kT[b][:, 3 * QS : S])
                nc.gpsimd.dma_start(out=qT_sb[3], in_=qT[b][:, 3 * QS : S])
                nc.gpsimd.dma_start(
                    out=v_sb[1],
                    in_=vA[b][S // 2 : S].rearrange("(t p) d -> p t d", p=128),
                )
                batch_tiles[b] = (kT_sb, qT_sb, v_sb)

            def emit_m2(b, qc, kt0, n_kt, p_tile, acc):
                _, _, v_sb = batch_tiles[b]
                for h in range(n_kt):
                    kt = kt0 + h
                    for j in range(4):
                        # start=True clears has_written for the WHOLE bank,
                        # so only the first slice of each packed bank may
                        # carry it; the second slice's first write lands on
                        # cleared bits and overwrites.
                        nc.tensor.matmul(
                            acc[j],
                            lhsT=p_tile[:, h, j * 128 : (j + 1) * 128],
                            rhs=v_sb[kt // 8][:, kt % 8, :],
                            start=(kt == 0),
                            stop=(kt == N_KT - 1),
                        )
                if kt0 + n_kt == N_KT:
                    emit_normalize(b, qc, acc)

            def emit_normalize(b, qc, acc):
                for j in range(4):
                    a = acc[j]
                    recip = o_pool.tile([128, 1], F32, tag="recip",
                                        name="recip")
                    nc.vector.reciprocal(recip, a[:, D : D + 1])
                    o_sb = o_pool.tile([128, D], F32, tag="o", name="o_sb")
                    nc.vector.tensor_scalar_mul(o_sb, a[:, 0:D], recip)
                    r0 = qc * QCHUNK + j * 128
                    nc.sync.dma_start(out=out[b, r0 : r0 + 128, :], in_=o_sb)

            # One continuous software pipeline across every (batch, q-chunk,
            # k-group): m2 for group g is emitted after m1 of group g+2, so
            # the in-order PE queue always has independent m1 work while exp
            # runs, with no pipeline drain at q-chunk or batch boundaries.
            pending = []
            load_batch(0)
            for b in range(BPC):
                for qc in range(N_QC):
                    kT_sb, qT_sb, _ = batch_tiles[b]
                    acc = [
                        psum_acc.tile(
                            [128, DA], F32, tag=f"acc{j}", name=f"acc{j}"
                        )
                        for j in range(4)
                    ]
                    first = b == 0 and qc == 0
                    groups = ([(0, 1), (1, 1)] + KT_GROUPS[1:]) if first \
                        else KT_GROUPS
                    for kt0, n_kt in groups:
                        if first and kt0 < 2:
                            ab = "A" if kt0 == 0 else "B"
                        else:
                            ab = "A" if (kt0 // 2) % 2 == 0 else "B"
                        s_psum = psum_s.tile(
                            [128, n_kt, QCHUNK], F32, tag=f"s{ab}",
                            name=f"s{ab}",
                        )
                        for h in range(n_kt):
                            kt = kt0 + h
                            nc.tensor.matmul(
                                s_psum[:, h, :],
                                lhsT=kT_sb[kt // 4][
                                    :, (kt % 4) * 128 : (kt % 4 + 1) * 128
                                ],
                                rhs=qT_sb[qc],
                                start=True,
                                stop=True,
                            )
                        p_tile = p_pool.tile(
                            [128, n_kt, QCHUNK], BF16, tag=f"p{ab}",
                            name=f"p{ab}",
                        )
                        nc.scalar.activation(
                            p_tile,
                            s_psum,
                            mybir.ActivationFunctionType.Exp,
                            scale=SCALE,
                        )
                        pending.append((b, qc, kt0, n_kt, p_tile, acc))
                        if len(pending) > 3:
                            emit_m2(*pending.pop(0))
                        # prefetch next batch's inputs once this batch's
                        # first q-chunk is underway
                        if b + 1 < BPC and qc == 1 and kt0 == 6:
                            load_batch(b + 1)
            for args in pending:
                emit_m2(*args)

    nc.compile()
    return nc


def _get_nc():
    if "nc" not in _CACHE:
        _CACHE["nc"] = _build()
    return _CACHE["nc"]


def kernel(query, key, value):
    global LAST_RESULTS
    bf16 = ml_dtypes.bfloat16
    q = np.ascontiguousarray(
        np.asarray(query, dtype=np.float32).transpose(0, 2, 1)
    ).astype(bf16)
    k = np.ascontiguousarray(
        np.asarray(key, dtype=np.float32).transpose(0, 2, 1)
    ).astype(bf16)
    v = np.asarray(value, dtype=np.float32)
    v_aug = np.concatenate(
        [v, np.ones((B, S, 1), dtype=np.float32)], axis=2
    ).astype(bf16)

    nc = _get_nc()
    in_maps = [
        {
            "qT": q[i * BPC : (i + 1) * BPC],
            "kT": k[i * BPC : (i + 1) * BPC],
            "vA": v_aug[i * BPC : (i + 1) * BPC],
        }
        for i in range(N_CORES)
    ]
    res = run_bass_kernel_spmd(
        nc, in_maps, core_ids=list(range(N_CORES)), trace=TRACE
    )
    LAST_RESULTS = res
    out = np.empty((B, S, D), dtype=np.float32)
    for i in range(N_CORES):
        out[i * BPC : (i + 1) * BPC] = res.results[i]["out"]
    return out
